# revision 10
# baseline (speedup 1.0000x reference)
"""Trainium2 Bass kernel for CoocOpModel.

out[b,s,z] = sum_{i,j} func[b,s,i] * cooc[i,j,z] * arg[b,s,j]
  with func = func_and_arg[..., :128], arg = func_and_arg[..., 128:]

Shapes (hardcoded): func_and_arg [4,1024,256] f32, cooccurrences [128,128,128] f32,
out [4,1024,128] f32.  D = 128, tokens T = 4096.

Strategy: data-parallel over tokens across 8 cores (512 tokens/core);
cooccurrence tensor replicated per core (fp16).

Per-core math, with t = local token index (512), i/j/z in [0,128):
  out_T[z, t] = sum_i  C_i^T @ G_i        (accumulated in one PSUM bank)
  C_i[j, z]   = cooc[i, j, z]             (stationary operand, fp16)
  G_i[j, t]   = arg_T[j, t] * func_T[i, t]  (moving operand, fp16)

For each i-group we need f_exp[j, (k,t)] = func_T[i0+k, t] replicated
across the 128 j-partitions; a DVE tensor-tensor multiply then builds G
and the per-i matmuls accumulate into PSUM.

The replication (D*D*T_core*2B = 16 MB/core) saturates the per-core DMA
fabric (~358 GB/s) if done purely as broadcast-DMA from DRAM, so it is
split:
  - 'd' groups: broadcast-DMA from DRAM (partition-step-0 source AP)
  - 'p' groups: PE ones-matmul (stationary ones[1,128], moving = f row
    chunks on a single SBUF partition) replicates a 512-wide f row into a
    PSUM bank; the DVE multiplies straight out of PSUM (fp32 in1, 1x
    instead of 2x, but no extra DMA bytes).  The broadcast matmuls are
    interleaved between the accumulating matmuls so the in-order PE queue
    never waits on them.
"""

import sys

sys.path.insert(0, "/opt/trn_rl_repo")

import numpy as np
from contextlib import ExitStack

import concourse.bass as bass
import concourse.tile as tile
from concourse import bacc, mybir
from concourse.bass_utils import run_bass_kernel_spmd

BF16 = mybir.dt.float16
F32 = mybir.dt.float32
NP_BF16 = np.float16

N_CORES = 8
D = 128
T_TOTAL = 4096
T_CORE = T_TOTAL // N_CORES  # 512

# Execution schedule: ('d'|'p', n_rows); sum of rows = 128.
# p-groups sit early/middle; head and tail are small d-groups.
SCHED = [
    ("d", 4), ("d", 4),
    ("p", 8), ("d", 8), ("d", 8), ("d", 8),
    ("p", 8), ("d", 8), ("d", 8), ("d", 8),
    ("p", 8), ("d", 8), ("d", 8), ("d", 8),
    ("d", 8), ("d", 8), ("d", 4), ("d", 4),
]
assert sum(r for _, r in SCHED) == D
PE_ROWS = sum(r for k, r in SCHED if k == "p")
I_PE0 = D - PE_ROWS  # p-groups own the top i-range [I_PE0, 128)

_NC_CACHE = None


def _build():
    nc = bacc.Bacc("TRN2", target_bir_lowering=False, debug=False, num_devices=N_CORES)

    f_t = nc.dram_tensor("f_t", [D, T_CORE], BF16, kind="ExternalInput").ap()
    a_t = nc.dram_tensor("a_t", [D, T_CORE], BF16, kind="ExternalInput").ap()
    # c2[j, i*128 + z] = cooc[i, j, z]
    c2 = nc.dram_tensor("c2", [D, D * D], BF16, kind="ExternalInput").ap()
    out_t = nc.dram_tensor("out_t", [D, T_CORE], F32, kind="ExternalOutput").ap()

    with tile.TileContext(nc) as tc:
        with ExitStack() as ctx:
            const_pool = ctx.enter_context(tc.tile_pool(name="const", bufs=1))
            fexp_pool = ctx.enter_context(tc.tile_pool(name="fexp", bufs=4))
            g_pool = ctx.enter_context(tc.tile_pool(name="g", bufs=3))
            out_pool = ctx.enter_context(tc.tile_pool(name="out", bufs=1))
            psum_pool = ctx.enter_context(
                tc.tile_pool(name="psum", bufs=1, space="PSUM")
            )
            bc_pool = ctx.enter_context(
                tc.tile_pool(name="bcps", bufs=5, space="PSUM")
            )

            a_sb = const_pool.tile([D, T_CORE], BF16, tag="a")
            nc.scalar.dma_start(a_sb[:], a_t[:, :])
            a_ap = a_sb[:]

            if PE_ROWS:
                ones_sb = const_pool.tile([1, D], BF16, tag="ones")
                nc.gpsimd.memset(ones_sb[:], 1.0)
                f_pe = const_pool.tile([1, PE_ROWS * T_CORE], BF16, tag="fpe")
                f_pe_src = bass.AP(
                    f_t.tensor, I_PE0 * T_CORE, [[0, 1], [1, PE_ROWS * T_CORE]]
                )
                nc.scalar.dma_start(f_pe[:], f_pe_src)

            ps = psum_pool.tile([D, T_CORE], F32)

            meta = []
            d_i0, p_i0 = 0, I_PE0
            for kind, sz in SCHED:
                if kind == "d":
                    meta.append((kind, sz, d_i0))
                    d_i0 += sz
                else:
                    meta.append((kind, sz, p_i0))
                    p_i0 += sz
            n_pos = len(meta)

            state = {"first": True}

            def real_mm(c_sb, gt, k, last):
                nc.tensor.matmul(
                    ps[:],
                    c_sb[:, k * D : (k + 1) * D],
                    gt[:, k * T_CORE : (k + 1) * T_CORE],
                    start=state["first"],
                    stop=last,
                )
                state["first"] = False

            def bc_mm(p, k):
                """Broadcast row i0+k of p-group at position p into a PSUM bank."""
                _, _, i0 = meta[p]
                off = (i0 - I_PE0) * T_CORE + k * T_CORE
                bc = bc_pool.tile([D, T_CORE], F32, tag="bc")
                nc.tensor.matmul(
                    bc[:],
                    ones_sb[:],
                    f_pe[:, off : off + T_CORE],
                    start=True,
                    stop=True,
                )
                return bc

            # bc tiles pending TT-consumption, keyed by position
            bc_tiles = {}

            dq = 0
            for p in range(n_pos):
                kind, sz, i0 = meta[p]
                nxt_pe = p + 1 < n_pos and meta[p + 1][0] == "p"

                # c tiles ride the otherwise-idle scalar queue so the two
                # fx queues never park a critical broadcast behind them
                c_sb = const_pool.tile([D, sz * D], BF16, tag=f"c{p}")
                nc.scalar.dma_start(c_sb[:], c2[:, i0 * D : (i0 + sz) * D])

                gt = g_pool.tile([D, sz * T_CORE], BF16, tag="g")

                if kind == "d":
                    fx = fexp_pool.tile([D, sz * T_CORE], BF16, tag="fxd")
                    if p == 0:
                        half = sz // 2
                        src_a = bass.AP(
                            f_t.tensor,
                            i0 * T_CORE,
                            [[0, D], [T_CORE, half], [1, T_CORE]],
                        )
                        src_b = bass.AP(
                            f_t.tensor,
                            (i0 + half) * T_CORE,
                            [[0, D], [T_CORE, half], [1, T_CORE]],
                        )
                        nc.sync.dma_start(fx[:, : half * T_CORE], src_a)
                        nc.gpsimd.dma_start(fx[:, half * T_CORE :], src_b)
                    else:
                        src = bass.AP(
                            f_t.tensor,
                            i0 * T_CORE,
                            [[0, D], [T_CORE, sz], [1, T_CORE]],
                        )
                        eng = nc.gpsimd if dq % 2 == 0 else nc.sync
                        eng.dma_start(fx[:], src)
                    dq += 1
                    a_view = bass.AP(
                        a_ap.tensor, a_ap.offset, [a_ap.ap[0], [0, sz], [1, T_CORE]]
                    )
                    nc.vector.tensor_mul(gt[:], a_view, fx[:])
                    # real mms; if the next position is a p-group, pre-issue
                    # its first 3 broadcast mms between our tail mms.
                    pre = 4 if nxt_pe else 0
                    lst = []
                    for k in range(sz):
                        real_mm(c_sb, gt, k, (p == n_pos - 1) and (k == sz - 1))
                        if pre and k >= sz - pre - 1 and len(lst) < pre:
                            lst.append(bc_mm(p + 1, len(lst)))
                    bc_tiles[p + 1] = lst
                else:
                    lst = bc_tiles.get(p, [])
                    # interleave: TT chunk k first (so pool reuse sees the
                    # read), then the bc mm for a later chunk, then real k.
                    for k in range(sz):
                        nc.vector.tensor_mul(
                            gt[:, k * T_CORE : (k + 1) * T_CORE],
                            a_ap,
                            lst[k][:],
                        )
                        if len(lst) < sz:
                            lst.append(bc_mm(p, len(lst)))
                        real_mm(c_sb, gt, k, False)

            # drain: split output halves across two copy engines + queues
            o_sb = out_pool.tile([D, T_CORE], F32, tag="o")
            h = T_CORE // 2
            nc.vector.tensor_copy(o_sb[:, :h], ps[:, :h])
            nc.sync.dma_start(out_t[:, :h], o_sb[:, :h])
            nc.scalar.copy(o_sb[:, h:], ps[:, h:])
            nc.scalar.dma_start(out_t[:, h:], o_sb[:, h:])

    nc.compile()
    return nc


def _get_nc():
    global _NC_CACHE
    if _NC_CACHE is None:
        _NC_CACHE = _build()
    return _NC_CACHE


def _prep_in_maps(func_and_arg, cooccurrences):
    fa = np.asarray(func_and_arg, dtype=np.float32).reshape(T_TOTAL, 2 * D)
    c2 = (
        np.ascontiguousarray(
            np.asarray(cooccurrences, dtype=np.float32).transpose(1, 0, 2)
        )
        .reshape(D, D * D)
        .astype(NP_BF16)
    )
    in_maps = []
    for c in range(N_CORES):
        s = fa[c * T_CORE : (c + 1) * T_CORE]  # [512, 256]
        f_tc = np.ascontiguousarray(s[:, :D].T).astype(NP_BF16)  # [128 i, 512 t]
        a_tc = np.ascontiguousarray(s[:, D:].T).astype(NP_BF16)  # [128 j, 512 t]
        in_maps.append({"f_t": f_tc, "a_t": a_tc, "c2": c2})
    return in_maps


def kernel(func_and_arg: np.ndarray, cooccurrences: np.ndarray) -> np.ndarray:
    assert func_and_arg.shape == (4, 1024, 2 * D)
    assert cooccurrences.shape == (D, D, D)

    in_maps = _prep_in_maps(func_and_arg, cooccurrences)
    nc = _get_nc()
    res = run_bass_kernel_spmd(nc, in_maps, core_ids=list(range(N_CORES)))

    # out_t per core: [z=128, t=512] -> [t, z]; concat over cores -> [4096, 128]
    outs = [res.results[c]["out_t"].T for c in range(N_CORES)]
    out = np.concatenate(outs, axis=0).reshape(4, 1024, D).astype(np.float32)
    return out


# revision 12
# speedup vs baseline: 1.0860x; 1.0860x over previous
"""Trainium2 Bass kernel for CoocOpModel.

out[b,s,z] = sum_{i,j} func[b,s,i] * cooc[i,j,z] * arg[b,s,j]
  with func = func_and_arg[..., :128], arg = func_and_arg[..., 128:]

Shapes (hardcoded): func_and_arg [4,1024,256] f32, cooccurrences [128,128,128] f32,
out [4,1024,128] f32.  D = 128, tokens T = 4096.

Strategy: data-parallel over tokens across 8 cores (512 tokens/core);
cooccurrence tensor replicated per core (fp16).

Per-core math, with t = local token index (512), i/j/z in [0,128):
  out_T[z, t] = sum_i  C_i^T @ G_i        (accumulated in one PSUM bank)
  C_i[j, z]   = cooc[i, j, z]             (stationary operand, fp16)
  G_i[j, t]   = arg_T[j, t] * func_T[i, t]  (moving operand, fp16)

For each i-group we need f_exp[j, (k,t)] = func_T[i0+k, t] replicated
across the 128 j-partitions; a DVE tensor-tensor multiply then builds G
and the per-i matmuls accumulate into PSUM.

The replication (D*D*T_core*2B = 16 MB/core) saturates the per-core DMA
fabric (~358 GB/s) if done purely as broadcast-DMA from DRAM, so it is
split:
  - 'd' groups: broadcast-DMA from DRAM (partition-step-0 source AP)
  - 'p' groups: PE ones-matmul (stationary ones[1,128], moving = f row
    chunks on a single SBUF partition) replicates a 512-wide f row into a
    PSUM bank; the DVE multiplies straight out of PSUM (fp32 in1, 1x
    instead of 2x, but no extra DMA bytes).  The broadcast matmuls are
    interleaved between the accumulating matmuls so the in-order PE queue
    never waits on them.
"""

import sys

sys.path.insert(0, "/opt/trn_rl_repo")

import numpy as np
from contextlib import ExitStack

import concourse.bass as bass
import concourse.tile as tile
from concourse import bacc, mybir
from concourse.bass_utils import run_bass_kernel_spmd

BF16 = mybir.dt.float16
F32 = mybir.dt.float32
NP_BF16 = np.float16

N_CORES = 8
D = 128
T_TOTAL = 4096
T_CORE = T_TOTAL // N_CORES  # 512

# Execution schedule: ('d'|'p', n_rows); sum of rows = 128.
# p-groups sit early/middle; head and tail are small d-groups.
SCHED = [
    ("d", 4), ("d", 4),
    ("p", 8), ("d", 8), ("d", 8), ("d", 8),
    ("p", 8), ("d", 8), ("d", 8), ("d", 8),
    ("p", 8), ("d", 8), ("d", 8), ("d", 8),
    ("d", 8), ("d", 8), ("d", 4), ("d", 4),
]
assert sum(r for _, r in SCHED) == D
PE_ROWS = sum(r for k, r in SCHED if k == "p")
I_PE0 = D - PE_ROWS  # p-groups own the top i-range [I_PE0, 128)

_NC_CACHE = None


def _build():
    nc = bacc.Bacc("TRN2", target_bir_lowering=False, debug=False, num_devices=N_CORES)

    f_t = nc.dram_tensor("f_t", [D, T_CORE], BF16, kind="ExternalInput").ap()
    a_t = nc.dram_tensor("a_t", [D, T_CORE], BF16, kind="ExternalInput").ap()
    # c2[j, i*128 + z] = cooc[i, j, z]
    c2 = nc.dram_tensor("c2", [D, D * D], BF16, kind="ExternalInput").ap()
    out_t = nc.dram_tensor("out_t", [D, T_CORE], F32, kind="ExternalOutput").ap()

    with tile.TileContext(nc) as tc:
        with ExitStack() as ctx:
            const_pool = ctx.enter_context(tc.tile_pool(name="const", bufs=1))
            fexp_pool = ctx.enter_context(tc.tile_pool(name="fexp", bufs=4))
            g_pool = ctx.enter_context(tc.tile_pool(name="g", bufs=3))
            out_pool = ctx.enter_context(tc.tile_pool(name="out", bufs=1))
            psum_pool = ctx.enter_context(
                tc.tile_pool(name="psum", bufs=1, space="PSUM")
            )
            bc_pool = ctx.enter_context(
                tc.tile_pool(name="bcps", bufs=5, space="PSUM")
            )

            a_sb = const_pool.tile([D, T_CORE], BF16, tag="a")
            nc.sync.dma_start(a_sb[:], a_t[:, :])
            a_ap = a_sb[:]

            if PE_ROWS:
                ones_sb = const_pool.tile([1, D], BF16, tag="ones")
                nc.gpsimd.memset(ones_sb[:], 1.0)
                f_pe = const_pool.tile([1, PE_ROWS * T_CORE], BF16, tag="fpe")
                f_pe_src = bass.AP(
                    f_t.tensor, I_PE0 * T_CORE, [[0, 1], [1, PE_ROWS * T_CORE]]
                )
                nc.scalar.dma_start(f_pe[:], f_pe_src)

            ps = psum_pool.tile([D, T_CORE], F32)

            meta = []
            d_i0, p_i0 = 0, I_PE0
            for kind, sz in SCHED:
                if kind == "d":
                    meta.append((kind, sz, d_i0))
                    d_i0 += sz
                else:
                    meta.append((kind, sz, p_i0))
                    p_i0 += sz
            n_pos = len(meta)

            state = {"first": True}

            def real_mm(c_sb, gt, k, last):
                nc.tensor.matmul(
                    ps[:],
                    c_sb[:, k * D : (k + 1) * D],
                    gt[:, k * T_CORE : (k + 1) * T_CORE],
                    start=state["first"],
                    stop=last,
                )
                state["first"] = False

            def bc_mm(p, k):
                """Broadcast row i0+k of p-group at position p into a PSUM bank."""
                _, _, i0 = meta[p]
                off = (i0 - I_PE0) * T_CORE + k * T_CORE
                bc = bc_pool.tile([D, T_CORE], F32, tag="bc")
                nc.tensor.matmul(
                    bc[:],
                    ones_sb[:],
                    f_pe[:, off : off + T_CORE],
                    start=True,
                    stop=True,
                )
                return bc

            # bc tiles pending TT-consumption, keyed by position
            bc_tiles = {}

            dq = 0
            for p in range(n_pos):
                kind, sz, i0 = meta[p]
                nxt_pe = p + 1 < n_pos and meta[p + 1][0] == "p"

                gt = g_pool.tile([D, sz * T_CORE], BF16, tag="g")

                if kind == "d":
                    fx = fexp_pool.tile([D, sz * T_CORE], BF16, tag="fxd")
                    if p == 0:
                        half = sz // 2
                        src_a = bass.AP(
                            f_t.tensor,
                            i0 * T_CORE,
                            [[0, D], [T_CORE, half], [1, T_CORE]],
                        )
                        src_b = bass.AP(
                            f_t.tensor,
                            (i0 + half) * T_CORE,
                            [[0, D], [T_CORE, half], [1, T_CORE]],
                        )
                        nc.scalar.dma_start(fx[:, : half * T_CORE], src_a)
                        nc.sync.dma_start(fx[:, half * T_CORE :], src_b)
                    else:
                        src = bass.AP(
                            f_t.tensor,
                            i0 * T_CORE,
                            [[0, D], [T_CORE, sz], [1, T_CORE]],
                        )
                        eng = nc.sync if dq % 2 == 0 else nc.scalar
                        eng.dma_start(fx[:], src)
                    dq += 1

                # fx first, then this position's c tile on the OPPOSITE queue
                c_sb = const_pool.tile([D, sz * D], BF16, tag=f"c{p}")
                ceng = nc.scalar if dq % 2 == 1 else nc.sync
                ceng.dma_start(c_sb[:], c2[:, i0 * D : (i0 + sz) * D])

                if kind == "d":
                    a_view = bass.AP(
                        a_ap.tensor, a_ap.offset, [a_ap.ap[0], [0, sz], [1, T_CORE]]
                    )
                    nc.vector.tensor_mul(gt[:], a_view, fx[:])
                    # real mms; if the next position is a p-group, pre-issue
                    # its first 3 broadcast mms between our tail mms.
                    pre = 4 if nxt_pe else 0
                    lst = []
                    for k in range(sz):
                        real_mm(c_sb, gt, k, (p == n_pos - 1) and (k == sz - 1))
                        if pre and k >= sz - pre - 1 and len(lst) < pre:
                            lst.append(bc_mm(p + 1, len(lst)))
                    bc_tiles[p + 1] = lst
                else:
                    lst = bc_tiles.get(p, [])
                    # interleave: TT chunk k first (so pool reuse sees the
                    # read), then the bc mm for a later chunk, then real k.
                    for k in range(sz):
                        nc.vector.tensor_mul(
                            gt[:, k * T_CORE : (k + 1) * T_CORE],
                            a_ap,
                            lst[k][:],
                        )
                        if len(lst) < sz:
                            lst.append(bc_mm(p, len(lst)))
                        real_mm(c_sb, gt, k, False)

            # drain: split output halves across two copy engines + queues
            o_sb = out_pool.tile([D, T_CORE], F32, tag="o")
            h = T_CORE // 2
            nc.vector.tensor_copy(o_sb[:, :h], ps[:, :h])
            nc.sync.dma_start(out_t[:, :h], o_sb[:, :h])
            nc.scalar.copy(o_sb[:, h:], ps[:, h:])
            nc.scalar.dma_start(out_t[:, h:], o_sb[:, h:])

    nc.compile()
    return nc


def _get_nc():
    global _NC_CACHE
    if _NC_CACHE is None:
        _NC_CACHE = _build()
    return _NC_CACHE


def _prep_in_maps(func_and_arg, cooccurrences):
    fa = np.asarray(func_and_arg, dtype=np.float32).reshape(T_TOTAL, 2 * D)
    c2 = (
        np.ascontiguousarray(
            np.asarray(cooccurrences, dtype=np.float32).transpose(1, 0, 2)
        )
        .reshape(D, D * D)
        .astype(NP_BF16)
    )
    in_maps = []
    for c in range(N_CORES):
        s = fa[c * T_CORE : (c + 1) * T_CORE]  # [512, 256]
        f_tc = np.ascontiguousarray(s[:, :D].T).astype(NP_BF16)  # [128 i, 512 t]
        a_tc = np.ascontiguousarray(s[:, D:].T).astype(NP_BF16)  # [128 j, 512 t]
        in_maps.append({"f_t": f_tc, "a_t": a_tc, "c2": c2})
    return in_maps


def kernel(func_and_arg: np.ndarray, cooccurrences: np.ndarray) -> np.ndarray:
    assert func_and_arg.shape == (4, 1024, 2 * D)
    assert cooccurrences.shape == (D, D, D)

    in_maps = _prep_in_maps(func_and_arg, cooccurrences)
    nc = _get_nc()
    res = run_bass_kernel_spmd(nc, in_maps, core_ids=list(range(N_CORES)))

    # out_t per core: [z=128, t=512] -> [t, z]; concat over cores -> [4096, 128]
    outs = [res.results[c]["out_t"].T for c in range(N_CORES)]
    out = np.concatenate(outs, axis=0).reshape(4, 1024, D).astype(np.float32)
    return out
